# revision 1
# baseline (speedup 1.0000x reference)
"""Trainium2 Bass kernel for nn_LittleBitParallelLinear.

Computes y = ((x * h_in) @ sign(V)) * s @ sign(U).T * h_out with
sign(z) = +1 for z >= 0, -1 otherwise.

Strategy: fold the whole weight chain into a single dense matrix on the
host:  W = diag(h_in) @ sign(V) @ diag(s) @ sign(U).T @ diag(h_out).
Because RANK == IN/2 == OUT/2, the folded matmul x @ W has exactly the
same FLOP count as the two-matmul form (IN*OUT == IN*RANK + RANK*OUT),
but the device kernel becomes a single streaming GEMM with no Sign
activations, no intermediate stage, and half the dependency depth.

Token-parallel across 8 NeuronCores: core i handles tokens
[i*1024, (i+1)*1024); W is replicated. Inside each core the compute is
transposed (tokens on the matmul free dim):

    yT = W.T @ xT    [OUT, TOK]  fp16, streamed to DRAM

x and W are fp16 (W pre-scaled by 1/4 for range headroom; the host
multiplies the output by 4). PSUM accumulates fp32; the fp32->fp16
output cast rides the scalar engine. DMA instruction count is kept low
(x in 4 chunks, W/y in 2-block chunks) — per-DMA fixed costs on the SP
queue are significant on this hardware — and all DMAs stay on the SP
queue (the Activation-engine DGE queue measures ~2x slower).
"""

import numpy as np

P = 128
TOKENS, IN, OUT = 8192, 4096, 4096
N_CORES = 8
TOK = TOKENS // N_CORES   # 1024 tokens per core
KI = IN // P              # 32 contraction subtiles
MO = OUT // P             # 32 output row-blocks
FREE = 512                # PSUM bank free-dim (fp32)
NT = TOK // FREE          # 2 free chunks
XCHUNK = 8                # k-subtiles per x DMA
WCHUNK = 2                # output blocks per W DMA (and per y store)
J = MO // WCHUNK

_cache = {}


def _build():
    import concourse.bacc as bacc
    import concourse.mybir as mybir
    import concourse.tile as tile

    f32 = mybir.dt.float32
    f16 = mybir.dt.float16
    Copy = mybir.ActivationFunctionType.Copy

    nc = bacc.Bacc("TRN2", target_bir_lowering=False, debug=False)

    xT = nc.dram_tensor("xT", [IN, TOK], f16, kind="ExternalInput").ap()
    # W pre-tiled on host: w[j, p, k, c, o] = W[k*128+p, (j*WCHUNK+c)*128+o]
    w_ = nc.dram_tensor(
        "w", [J, P, KI, WCHUNK, P], f16, kind="ExternalInput"
    ).ap()
    yT = nc.dram_tensor("yT", [OUT, TOK], f16, kind="ExternalOutput").ap()

    with tile.TileContext(nc) as tc:
        with (
            tc.tile_pool(name="x", bufs=1) as xpool,
            tc.tile_pool(name="w", bufs=4) as wpool,
            tc.tile_pool(name="y", bufs=4) as ypool,
            tc.tile_pool(name="ps", bufs=8, space="PSUM") as psum,
        ):
            xs = xpool.tile([P, KI, TOK], f16)
            x3 = xT.rearrange("(k p) t -> p k t", p=P)
            y3 = yT.rearrange("(m p) t -> p m t", p=P)

            w_tiles = {}

            def load_w(j):
                wt = wpool.tile(
                    [P, KI, WCHUNK, P], f16, name=f"wt{j}", tag="wt"
                )
                nc.sync.dma_start(wt, w_[j])
                w_tiles[j] = wt

            load_w(0)
            next_wj = 1
            for kc in range(0, KI, XCHUNK):
                nc.sync.dma_start(
                    xs[:, kc : kc + XCHUNK], x3[:, kc : kc + XCHUNK]
                )
                if kc == 0 and next_wj < J:
                    load_w(next_wj)
                    next_wj += 1

            # Pair-major: k-major across the WCHUNK output blocks of each
            # W chunk (4 PSUM banks per pair, 2 pairs rotating through 8
            # banks) so the PE has 2x the runnable matmuls per x chunk
            # while x is still streaming in.
            for j in range(J):
                if next_wj <= min(j + 2, J - 1):
                    load_w(next_wj)
                    next_wj += 1
                wt = w_tiles[j]
                pss = {
                    (c, n): psum.tile(
                        [P, FREE], f32, name=f"ps_{j}_{c}_{n}", tag="ps"
                    )
                    for c in range(WCHUNK)
                    for n in range(NT)
                }
                for k in range(KI):
                    for c in range(WCHUNK):
                        for n in range(NT):
                            nc.tensor.matmul(
                                pss[(c, n)],
                                lhsT=wt[:, k, c],
                                rhs=xs[:, k, n * FREE : (n + 1) * FREE],
                                start=(k == 0),
                                stop=(k == KI - 1),
                            )
                yt = ypool.tile(
                    [P, WCHUNK, TOK], f16, name=f"yt_{j}", tag="yt"
                )
                last = j == J - 1
                for c in range(WCHUNK):
                    for n in range(NT):
                        nc.scalar.activation(
                            yt[:, c, n * FREE : (n + 1) * FREE],
                            pss[(c, n)],
                            Copy,
                        )
                    if last:
                        nc.sync.dma_start(
                            y3[:, WCHUNK * j + c : WCHUNK * j + c + 1],
                            yt[:, c : c + 1],
                        )
                if not last:
                    nc.sync.dma_start(
                        y3[:, WCHUNK * j : WCHUNK * (j + 1)], yt
                    )
                w_tiles.pop(j)

    nc.compile()
    return nc


def _run(inputs, trace=False):
    from concourse.bass_utils import run_bass_kernel_spmd

    if "nc" not in _cache:
        _cache["nc"] = _build()
    nc = _cache["nc"]

    x = np.asarray(inputs["x"], dtype=np.float32)
    u = np.asarray(inputs["u"], dtype=np.float32)
    v = np.asarray(inputs["v"], dtype=np.float32)
    s = np.asarray(inputs["s"], dtype=np.float32)
    h_in = np.asarray(inputs["h_in"], dtype=np.float32)
    h_out = np.asarray(inputs["h_out"], dtype=np.float32)

    bu = np.where(u >= 0, np.float32(1.0), np.float32(-1.0))
    bv = np.where(v >= 0, np.float32(1.0), np.float32(-1.0))
    W = (bv * s[None, :]) @ bu.T                 # [IN, OUT]
    W *= h_in[:, None]
    W *= h_out[None, :]
    W *= np.float32(0.25)                        # fp16 range headroom
    # w[j, p, k, c, o] = W[k*128+p, (j*WCHUNK+c)*128+o]
    w_t = np.ascontiguousarray(
        W.reshape(KI, P, J, WCHUNK, P).transpose(2, 1, 0, 3, 4)
    ).astype(np.float16)

    in_maps = []
    for i in range(N_CORES):
        xT_i = np.ascontiguousarray(x[i * TOK : (i + 1) * TOK, :].T).astype(
            np.float16
        )
        in_maps.append({"xT": xT_i, "w": w_t})

    _cache["in_maps"] = in_maps
    res = run_bass_kernel_spmd(
        nc, in_maps, core_ids=list(range(N_CORES)), trace=trace
    )

    y = np.empty((TOKENS, OUT), dtype=np.float32)
    for i in range(N_CORES):
        y[i * TOK : (i + 1) * TOK, :] = res.results[i]["yT"].T.astype(np.float32)
    y *= np.float32(4.0)
    return y, res


def kernel(**inputs):
    y, _ = _run(inputs, trace=False)
    return y

